# revision 8
# baseline (speedup 1.0000x reference)
"""Trainium2 Bass kernel for multi-head self-attention with RoPE (causal).

Problem shape (hardcoded): x [1, 4096, 1024], 16 heads, d_k=64, fp32.
Sharding: tensor-parallel over heads -- 2 heads per NeuronCore, 8 cores.
Each core computes Q/K/V projections for its 2 heads, RoPE, causal
attention, and a full [4096, 1024] partial of the output projection
(columns of wo matching its heads). Partials are summed on the host
(the all-reduce of row-parallel linear, done at unshard time).

Layout choices inside a core:
  - x is fed pre-transposed (xT [1024, 4096]) so all projection matmuls
    contract d on the partition axis without on-device transposes.
  - Q,K live as [128, 4096] = (2 heads x 64 dims) x seq, i.e. transposed.
    The even/odd RoPE interleave is pre-permuted into the wq/wk rows on
    the host so rotation pairs become contiguous 32-row blocks.
  - Scores are computed transposed, ST[k, q], so the post-exp P matrix
    feeds P@V directly as the moving operand with k on partitions.
  - V is computed in [seq, d] layout with an extra ones column per head:
    the P@V matmul then yields softmax numerators AND denominators.
  - All matmuls use float32r (1-pass fp22 multiply, fp32 accumulate).
"""

import os
import numpy as np

S = 4096
D = 1024
P = 128
DK = 64
SC = 512          # q-chunk width for attention
NQ = S // SC      # 8
NSUB = SC // P    # 4
NKC = S // P      # 32
PO = D // P       # 8 contraction chunks for projections
NCORES = 8
THETA = 10000.0

LAST_EXEC_NS = None
LAST_RESULTS = None

_cache = {}


def _build_bass():
    import concourse.bacc as bacc
    import concourse.tile as tile
    from concourse import mybir

    F32 = mybir.dt.float32
    F32R = mybir.dt.float32r
    EXP = mybir.ActivationFunctionType.Exp
    LN = mybir.ActivationFunctionType.Ln
    MULT = mybir.AluOpType.mult
    ADD = mybir.AluOpType.add

    nc = bacc.Bacc("TRN2", target_bir_lowering=False, debug=False)

    xT = nc.dram_tensor("xT", [D, S], F32R, kind="ExternalInput")
    wqT = nc.dram_tensor("wqT", [D, P], F32R, kind="ExternalInput")
    wkT = nc.dram_tensor("wkT", [D, P], F32R, kind="ExternalInput")
    wvT = nc.dram_tensor("wvT", [D, P], F32R, kind="ExternalInput")
    woT = nc.dram_tensor("woT", [P, D], F32R, kind="ExternalInput")
    t1 = nc.dram_tensor("t1", [P, S], F32, kind="ExternalInput")
    t2s = nc.dram_tensor("t2s", [P, S], F32, kind="ExternalInput")
    trimask = nc.dram_tensor("trimask", [P, P], F32, kind="ExternalInput")
    swapmat = nc.dram_tensor("swapmat", [P, P], F32R, kind="ExternalInput")
    out = nc.dram_tensor("out", [S, D], F32, kind="ExternalOutput")

    xT_t = xT.ap().rearrange("(po pi) s -> pi po s", pi=P)
    wqT_t = wqT.ap().rearrange("(po pi) m -> pi po m", pi=P)
    wkT_t = wkT.ap().rearrange("(po pi) m -> pi po m", pi=P)
    wvT_t = wvT.ap().rearrange("(po pi) m -> pi po m", pi=P)

    with tile.TileContext(nc) as tc:
        with (
            tc.tile_pool(name="persist", bufs=1) as pp,
            tc.tile_pool(name="weights", bufs=1) as wp,
        ):
            QTr = pp.tile([P, S], F32R, tag="qtr")
            KTr = pp.tile([P, S], F32R, tag="ktr")
            Vp = pp.tile([P, NKC, 130], F32R, tag="vp")
            oT0 = pp.tile([DK, S], F32R, tag="ot0")
            oT1 = pp.tile([DK, S], F32R, tag="ot1")

            wq_sb = wp.tile([P, PO, P], F32R, tag="wq")
            wk_sb = wp.tile([P, PO, P], F32R, tag="wk")
            wv_sb = wp.tile([P, PO, P], F32R, tag="wv")
            wo0_sb = wp.tile([DK, D], F32R, tag="wo0")
            wo1_sb = wp.tile([DK, D], F32R, tag="wo1")
            tri_sb = wp.tile([P, P], F32, tag="tri")
            swap_sb = wp.tile([P, P], F32R, tag="swap")

            nc.sync.dma_start(wq_sb[:], wqT_t)
            nc.sync.dma_start(wk_sb[:], wkT_t)
            nc.sync.dma_start(wv_sb[:], wvT_t)
            nc.sync.dma_start(wo0_sb[:], woT.ap()[0:DK, :])
            nc.sync.dma_start(wo1_sb[:], woT.ap()[DK:P, :])
            nc.sync.dma_start(tri_sb[:], trimask.ap())
            nc.sync.dma_start(swap_sb[:], swapmat.ap())

            # ones columns for the softmax-denominator trick
            # (memset can't write f32r -> memset an f32 tile, copy-convert)
            ones_sb = wp.tile([P, NKC], F32, tag="ones")
            nc.vector.memset(ones_sb[:], 1.0)
            nc.vector.tensor_copy(Vp[:, :, 64], ones_sb[:])
            nc.vector.tensor_copy(Vp[:, :, 129], ones_sb[:])

            # ---------------- Phase 1: projections + RoPE + V ----------------
            with (
                tc.tile_pool(name="ps1", bufs=1, space="PSUM") as ps1,
                tc.tile_pool(name="psv", bufs=2, space="PSUM") as psv_pool,
                tc.tile_pool(name="xchunk", bufs=2) as xpool,
                tc.tile_pool(name="tchunk", bufs=2) as tpool,
                tc.tile_pool(name="rope", bufs=2) as rpool,
            ):
                for j in range(NQ):
                    sl = slice(j * SC, (j + 1) * SC)
                    xt = xpool.tile([P, PO, SC], F32R, tag="xt")
                    nc.sync.dma_start(xt[:], xT_t[:, :, sl])
                    t1t = tpool.tile([P, SC], F32, tag="t1")
                    nc.sync.dma_start(t1t[:], t1.ap()[:, sl])
                    t2t = tpool.tile([P, SC], F32, tag="t2")
                    nc.sync.dma_start(t2t[:], t2s.ap()[:, sl])

                    for w_sb, dest, tagp in ((wq_sb, QTr, "q"), (wk_sb, KTr, "k")):
                        psq = ps1.tile([P, SC], F32, tag=tagp)
                        for po in range(PO):
                            nc.tensor.matmul(
                                psq[:], w_sb[:, po, :], xt[:, po, :],
                                start=(po == 0), stop=(po == PO - 1),
                            )
                        # RoPE: dest = t1*psq + swap(t2s*psq)
                        b = rpool.tile([P, SC], F32R, tag="b")
                        nc.vector.tensor_tensor(b[:], t2t[:], psq[:], MULT)
                        pssw = ps1.tile([P, SC], F32, tag="sw")
                        nc.tensor.matmul(pssw[:], swap_sb[:], b[:], start=True, stop=True)
                        a1 = rpool.tile([P, SC], F32, tag="a1")
                        nc.vector.tensor_tensor(a1[:], t1t[:], psq[:], MULT)
                        nc.vector.tensor_tensor(dest[:, sl], a1[:], pssw[:], ADD)

                    for m in range(NSUB):
                        kc = j * NSUB + m
                        psv = psv_pool.tile([P, P], F32, tag="v")
                        for po in range(PO):
                            nc.tensor.matmul(
                                psv[:], xt[:, po, m * P:(m + 1) * P], wv_sb[:, po, :],
                                start=(po == 0), stop=(po == PO - 1),
                            )
                        nc.any.tensor_copy(Vp[:, kc, 0:64], psv[0:P, 0:64])
                        nc.any.tensor_copy(Vp[:, kc, 65:129], psv[0:P, 64:128])

            # ---------------- Phase 2+3: attention + projection ----------------
            with (
                tc.tile_pool(name="pst", bufs=3, space="PSUM") as pst,
                tc.tile_pool(name="pso", bufs=2, space="PSUM") as pso,
                tc.tile_pool(name="psr", bufs=2, space="PSUM") as psr,
                tc.tile_pool(name="ppool", bufs=3) as ppool,
                tc.tile_pool(name="small", bufs=2) as small,
                tc.tile_pool(name="rbp", bufs=2) as rbp,
                tc.tile_pool(name="rtp", bufs=2) as rtp,
            ):
                for jq in range(NQ):
                    qsl = slice(jq * SC, (jq + 1) * SC)
                    for h in range(2):
                        hp = slice(DK * h, DK * (h + 1))
                        oT_h = oT0 if h == 0 else oT1
                        pso_t = pso.tile([65, SC], F32, tag="o")
                        nkc = NSUB * (jq + 1)
                        for kc in range(nkc):
                            t = kc - NSUB * jq  # >= 0 -> diagonal straddle
                            col0 = t * P if t >= 0 else 0
                            ps_s = pst.tile([P, SC], F32, tag="st")
                            nc.tensor.matmul(
                                ps_s[:, col0:SC],
                                KTr[hp, kc * P:(kc + 1) * P],
                                QTr[hp, jq * SC + col0:(jq + 1) * SC],
                                start=True, stop=True,
                            )
                            pt = ppool.tile([P, SC], F32R, tag="p")
                            nc.scalar.activation(
                                pt[:, col0:SC], ps_s[:, col0:SC], EXP, scale=0.125
                            )
                            if t >= 0:
                                # diagonal block: zero keys above the diagonal
                                nc.vector.tensor_tensor(
                                    pt[:, col0:col0 + P], pt[:, col0:col0 + P],
                                    tri_sb[:], MULT,
                                )
                            nc.tensor.matmul(
                                pso_t[:, col0:SC],
                                Vp[:, kc, 65 * h:65 * h + 65],
                                pt[:, col0:SC],
                                start=(kc == 0), stop=(kc == nkc - 1),
                            )
                        # softmax denominators: recip via Ln -> Exp(-x), broadcast
                        ln_t = small.tile([1, SC], F32, tag="ln")
                        nc.scalar.activation(ln_t[:], pso_t[64:65, :], LN)
                        rr_t = small.tile([1, SC], F32, tag="rr")
                        nc.scalar.activation(rr_t[:], ln_t[:], EXP, scale=-1.0)
                        rb_t = rbp.tile([DK, SC], F32, tag="rb")
                        nc.gpsimd.partition_broadcast(rb_t[:], rr_t[:])
                        nc.vector.tensor_tensor(
                            oT_h[:, qsl], pso_t[0:DK, :], rb_t[:], MULT
                        )
                    # output projection for the finished q-chunk
                    for m in range(NSUB):
                        sc = jq * NSUB + m
                        ssl = slice(sc * P, (sc + 1) * P)
                        rt = rtp.tile([P, D], F32, tag="rt")
                        for jn in range(2):
                            psr_t = psr.tile([P, SC], F32, tag="r")
                            nc.tensor.matmul(
                                psr_t[:], oT0[:, ssl], wo0_sb[:, jn * SC:(jn + 1) * SC],
                                start=True, stop=False,
                            )
                            nc.tensor.matmul(
                                psr_t[:], oT1[:, ssl], wo1_sb[:, jn * SC:(jn + 1) * SC],
                                start=False, stop=True,
                            )
                            nc.any.tensor_copy(rt[:, jn * SC:(jn + 1) * SC], psr_t[:])
                        nc.sync.dma_start(out.ap()[ssl, :], rt[:])

    nc.compile()
    return nc


def _rope_tables():
    inv_freq = 1.0 / (THETA ** (np.arange(0, DK, 2, dtype=np.float64) / DK))  # [32]
    pos = np.arange(S, dtype=np.float64)
    freqs = pos[:, None] * inv_freq[None, :]      # [S, 32]
    cosT = np.cos(freqs).T.astype(np.float32)     # [32, S]
    sinT = np.sin(freqs).T.astype(np.float32)
    # t1 rows (per 64-block): [cos; cos];   t2s rows: [+sin; -sin]
    t1 = np.tile(np.concatenate([cosT, cosT], axis=0), (2, 1))      # [128, S]
    t2s_ = np.tile(np.concatenate([sinT, -sinT], axis=0), (2, 1))   # [128, S]
    return np.ascontiguousarray(t1), np.ascontiguousarray(t2s_)


def _host_prep(x, wq, wk, wv, wo):
    x2 = np.asarray(x, dtype=np.float32).reshape(S, D)
    xT = np.ascontiguousarray(x2.T)

    # even/odd de-interleave permutation within each head's 64 rows
    perm64 = np.concatenate([np.arange(0, DK, 2), np.arange(1, DK, 2)])
    perm128 = np.concatenate([perm64, perm64 + DK])

    t1, t2s_ = _rope_tables()
    trimask = np.triu(np.ones((P, P), dtype=np.float32))
    swp = np.zeros((P, P), dtype=np.float32)
    for b in range(2):
        for i in range(32):
            swp[b * 64 + i, b * 64 + 32 + i] = 1.0
            swp[b * 64 + 32 + i, b * 64 + i] = 1.0

    wq = np.asarray(wq, dtype=np.float32)
    wk = np.asarray(wk, dtype=np.float32)
    wv = np.asarray(wv, dtype=np.float32)
    wo = np.asarray(wo, dtype=np.float32)

    in_maps = []
    for c in range(NCORES):
        rows = slice(P * c, P * (c + 1))
        wq_c = wq[rows][perm128]
        wk_c = wk[rows][perm128]
        wv_c = wv[rows]
        in_maps.append({
            "xT": xT,
            "wqT": np.ascontiguousarray(wq_c.T),
            "wkT": np.ascontiguousarray(wk_c.T),
            "wvT": np.ascontiguousarray(wv_c.T),
            "woT": np.ascontiguousarray(wo[:, rows].T),
            "t1": t1,
            "t2s": t2s_,
            "trimask": trimask,
            "swapmat": swp,
        })
    return in_maps


def _install_ntff_hook():
    """Register the axon NTFF profiling hook (missing antenv.axon_hooks shim)."""
    import sys
    import types
    import importlib

    try:
        import antenv.axon_hooks  # noqa: F401
        return
    except ImportError:
        pass
    try:
        import antenv
        boot = importlib.import_module("trn_agent_boot.trn_boot")
        mod = types.ModuleType("antenv.axon_hooks")
        state = {"hook": None}
        mod.set_axon_ntff_profile_hook = lambda h: state.update(hook=h)
        mod.get_axon_ntff_profile_hook = lambda: state["hook"]
        sys.modules["antenv.axon_hooks"] = mod
        antenv.axon_hooks = mod
        hook = boot._ntff_profile_via_ctypes("/opt/axon/libaxon_pjrt.so")
        mod.set_axon_ntff_profile_hook(hook)
    except Exception as e:  # profiling is best-effort
        print(f"ntff hook install failed: {e}")


def kernel(x, wq, wk, wv, wo):
    global LAST_EXEC_NS, LAST_RESULTS
    from concourse import bass_utils

    trace_requested = bool(int(os.environ.get("TRN_TRACE", "0")))
    if trace_requested:
        _install_ntff_hook()
        # artifact upload needs remote storage; stub it out in this sandbox
        bass_utils.upload_artifacts = lambda tmpdir: "local://" + str(tmpdir)

    if "nc" not in _cache:
        _cache["nc"] = _build_bass()
    nc = _cache["nc"]

    in_maps = _host_prep(x, wq, wk, wv, wo)
    res = bass_utils.run_bass_kernel_spmd(
        nc, in_maps, core_ids=list(range(NCORES)), trace=trace_requested
    )
    LAST_EXEC_NS = res.exec_time_ns
    LAST_RESULTS = res
    acc = np.zeros((S, D), dtype=np.float32)
    for r in res.results:
        acc += np.asarray(r["out"], dtype=np.float32)
    return acc.reshape(1, S, D)


# revision 9
# speedup vs baseline: 1.4602x; 1.4602x over previous
"""Trainium2 Bass kernel for multi-head self-attention with RoPE (causal).

Problem shape (hardcoded): x [1, 4096, 1024], 16 heads, d_k=64, fp32.
Sharding: tensor-parallel over heads -- 2 heads per NeuronCore, 8 cores.
Each core computes Q/K/V projections for its 2 heads, RoPE, causal
attention, and a full [4096, 1024] partial of the output projection
(columns of wo matching its heads). Partials are summed on the host
(the all-reduce of row-parallel linear, done at unshard time).

Layout choices inside a core:
  - x is fed pre-transposed (xT [1024, 4096]) so all projection matmuls
    contract d on the partition axis without on-device transposes.
  - Q,K live as [128, 4096] = (2 heads x 64 dims) x seq, i.e. transposed.
    The even/odd RoPE interleave is pre-permuted into the wq/wk rows on
    the host so rotation pairs become contiguous 32-row blocks.
  - Scores are computed transposed, ST[k, q], so the post-exp P matrix
    feeds P@V directly as the moving operand with k on partitions.
  - V is computed in [seq, d] layout with an extra ones column per head:
    the P@V matmul then yields softmax numerators AND denominators.
  - All matmuls use float32r (1-pass fp22 multiply, fp32 accumulate).
"""

import os
import numpy as np

S = 4096
D = 1024
P = 128
DK = 64
SC = 512          # q-chunk width for attention
NQ = S // SC      # 8
NSUB = SC // P    # 4
NKC = S // P      # 32
PO = D // P       # 8 contraction chunks for projections
NCORES = 8
THETA = 10000.0

LAST_EXEC_NS = None
LAST_RESULTS = None

_cache = {}


def _build_bass():
    import concourse.bacc as bacc
    import concourse.tile as tile
    from concourse import mybir

    F32 = mybir.dt.float32
    F32R = mybir.dt.float32r
    EXP = mybir.ActivationFunctionType.Exp
    LN = mybir.ActivationFunctionType.Ln
    MULT = mybir.AluOpType.mult
    ADD = mybir.AluOpType.add

    nc = bacc.Bacc("TRN2", target_bir_lowering=False, debug=False)

    xT = nc.dram_tensor("xT", [D, S], F32R, kind="ExternalInput")
    wqT = nc.dram_tensor("wqT", [D, P], F32R, kind="ExternalInput")
    wkT = nc.dram_tensor("wkT", [D, P], F32R, kind="ExternalInput")
    wvT = nc.dram_tensor("wvT", [D, P], F32R, kind="ExternalInput")
    woT = nc.dram_tensor("woT", [P, D], F32R, kind="ExternalInput")
    t1 = nc.dram_tensor("t1", [P, S], F32, kind="ExternalInput")
    t2s = nc.dram_tensor("t2s", [P, S], F32, kind="ExternalInput")
    trimask = nc.dram_tensor("trimask", [P, P], F32, kind="ExternalInput")
    swapmat = nc.dram_tensor("swapmat", [P, P], F32R, kind="ExternalInput")
    out = nc.dram_tensor("out", [S, D], F32, kind="ExternalOutput")

    xT_t = xT.ap().rearrange("(po pi) s -> pi po s", pi=P)
    wqT_t = wqT.ap().rearrange("(po pi) m -> pi po m", pi=P)
    wkT_t = wkT.ap().rearrange("(po pi) m -> pi po m", pi=P)
    wvT_t = wvT.ap().rearrange("(po pi) m -> pi po m", pi=P)

    with tile.TileContext(nc) as tc:
        with (
            tc.tile_pool(name="persist", bufs=1) as pp,
            tc.tile_pool(name="weights", bufs=1) as wp,
        ):
            QTr = pp.tile([P, S], F32R, tag="qtr")
            KTr0 = pp.tile([P, S], F32R, tag="ktr0")
            KTr1 = pp.tile([P, S], F32R, tag="ktr1")
            Vp = pp.tile([P, NKC, 130], F32R, tag="vp")
            oT = pp.tile([P, S], F32R, tag="ot")

            wq_sb = wp.tile([P, PO, P], F32R, tag="wq")
            wk_sb = wp.tile([P, PO, P], F32R, tag="wk")
            wv_sb = wp.tile([P, PO, P], F32R, tag="wv")
            wo_sb = wp.tile([P, D], F32R, tag="wo")
            tri_sb = wp.tile([P, P], F32, tag="tri")
            swap_sb = wp.tile([P, P], F32R, tag="swap")

            nc.sync.dma_start(wq_sb[:], wqT_t)
            nc.sync.dma_start(wk_sb[:], wkT_t)
            nc.sync.dma_start(wv_sb[:], wvT_t)
            nc.sync.dma_start(wo_sb[:], woT.ap())
            nc.sync.dma_start(tri_sb[:], trimask.ap())
            nc.sync.dma_start(swap_sb[:], swapmat.ap())

            # ones columns for the softmax-denominator trick
            # (memset can't write f32r -> memset an f32 tile, copy-convert)
            ones_sb = wp.tile([P, NKC], F32, tag="ones")
            nc.vector.memset(ones_sb[:], 1.0)
            nc.vector.memset(KTr0.bitcast(F32)[DK:P, :], 0.0)
            nc.vector.memset(KTr1.bitcast(F32)[0:DK, :], 0.0)
            nc.vector.tensor_copy(Vp[:, :, 64], ones_sb[:])
            nc.vector.tensor_copy(Vp[:, :, 129], ones_sb[:])

            # ---------------- Phase 1: projections + RoPE + V ----------------
            with (
                tc.tile_pool(name="ps1", bufs=1, space="PSUM") as ps1,
                tc.tile_pool(name="psv", bufs=2, space="PSUM") as psv_pool,
                tc.tile_pool(name="xchunk", bufs=2) as xpool,
                tc.tile_pool(name="tchunk", bufs=2) as tpool,
                tc.tile_pool(name="rope", bufs=2) as rpool,
            ):
                for j in range(NQ):
                    sl = slice(j * SC, (j + 1) * SC)
                    xt = xpool.tile([P, PO, SC], F32R, tag="xt")
                    nc.sync.dma_start(xt[:], xT_t[:, :, sl])
                    t1t = tpool.tile([P, SC], F32, tag="t1")
                    nc.sync.dma_start(t1t[:], t1.ap()[:, sl])
                    t2t = tpool.tile([P, SC], F32, tag="t2")
                    nc.sync.dma_start(t2t[:], t2s.ap()[:, sl])

                    for w_sb, dest, tagp in ((wq_sb, QTr, "q"), (wk_sb, None, "k")):
                        psq = ps1.tile([P, SC], F32, tag=tagp)
                        for po in range(PO):
                            nc.tensor.matmul(
                                psq[:], w_sb[:, po, :], xt[:, po, :],
                                start=(po == 0), stop=(po == PO - 1),
                            )
                        # RoPE: dest = t1*psq + swap(t2s*psq)
                        b = rpool.tile([P, SC], F32R, tag="b")
                        nc.vector.tensor_tensor(b[:], t2t[:], psq[:], MULT)
                        pssw = ps1.tile([P, SC], F32, tag="sw")
                        nc.tensor.matmul(pssw[:], swap_sb[:], b[:], start=True, stop=True)
                        a1 = rpool.tile([P, SC], F32, tag="a1")
                        nc.vector.tensor_tensor(a1[:], t1t[:], psq[:], MULT)
                        if dest is not None:
                            nc.vector.tensor_tensor(dest[:, sl], a1[:], pssw[:], ADD)
                        else:
                            nc.vector.tensor_tensor(
                                KTr0[0:DK, sl], a1[0:DK, :], pssw[0:DK, :], ADD)
                            nc.vector.tensor_tensor(
                                KTr1[DK:P, sl], a1[DK:P, :], pssw[DK:P, :], ADD)

                    for m in range(NSUB):
                        kc = j * NSUB + m
                        psv = psv_pool.tile([P, P], F32, tag="v")
                        for po in range(PO):
                            nc.tensor.matmul(
                                psv[:], xt[:, po, m * P:(m + 1) * P], wv_sb[:, po, :],
                                start=(po == 0), stop=(po == PO - 1),
                            )
                        nc.any.tensor_copy(Vp[:, kc, 0:64], psv[0:P, 0:64])
                        nc.any.tensor_copy(Vp[:, kc, 65:129], psv[0:P, 64:128])

            # ---------------- Phase 2+3: attention + projection ----------------
            with (
                tc.tile_pool(name="pst", bufs=3, space="PSUM") as pst,
                tc.tile_pool(name="pso", bufs=3, space="PSUM") as pso,
                tc.tile_pool(name="psr", bufs=2, space="PSUM") as psr,
                tc.tile_pool(name="ppool", bufs=3) as ppool,
                tc.tile_pool(name="small", bufs=2) as small,
                tc.tile_pool(name="rbp", bufs=2) as rbp,
                tc.tile_pool(name="rtp", bufs=2) as rtp,
            ):
                for jq in range(NQ):
                    qsl = slice(jq * SC, (jq + 1) * SC)
                    for h in range(2):
                        KTr_h = KTr0 if h == 0 else KTr1
                        pso_t = pso.tile([65, SC], F32, tag="o")
                        nkc = NSUB * (jq + 1)
                        for kc in range(nkc):
                            t = kc - NSUB * jq  # >= 0 -> diagonal straddle
                            col0 = t * P if t >= 0 else 0
                            ps_s = pst.tile([P, SC], F32, tag="st")
                            nc.tensor.matmul(
                                ps_s[:, col0:SC],
                                KTr_h[:, kc * P:(kc + 1) * P],
                                QTr[:, jq * SC + col0:(jq + 1) * SC],
                                start=True, stop=True,
                            )
                            pt = ppool.tile([P, SC], F32R, tag="p")
                            nc.scalar.activation(
                                pt[:, col0:SC], ps_s[:, col0:SC], EXP, scale=0.125
                            )
                            if t >= 0:
                                # diagonal block: zero keys above the diagonal
                                nc.vector.tensor_tensor(
                                    pt[:, col0:col0 + P], pt[:, col0:col0 + P],
                                    tri_sb[:], MULT,
                                )
                            nc.tensor.matmul(
                                pso_t[:, col0:SC],
                                Vp[:, kc, 65 * h:65 * h + 65],
                                pt[:, col0:SC],
                                start=(kc == 0), stop=(kc == nkc - 1),
                            )
                        # softmax denominators: recip via Ln -> Exp(-x), broadcast
                        ln_t = small.tile([1, SC], F32, tag="ln")
                        nc.scalar.activation(ln_t[:], pso_t[64:65, :], LN)
                        rr_t = small.tile([1, SC], F32, tag="rr")
                        nc.scalar.activation(rr_t[:], ln_t[:], EXP, scale=-1.0)
                        rb_t = rbp.tile([DK, SC], F32, tag="rb")
                        nc.gpsimd.partition_broadcast(rb_t[:], rr_t[:])
                        nc.vector.tensor_tensor(
                            oT[DK * h:DK * (h + 1), qsl], pso_t[0:DK, :], rb_t[:], MULT
                        )
                    # output projection for the finished q-chunk
                    for m in range(NSUB):
                        sc = jq * NSUB + m
                        ssl = slice(sc * P, (sc + 1) * P)
                        rt = rtp.tile([P, D], F32, tag="rt")
                        for jn in range(2):
                            psr_t = psr.tile([P, SC], F32, tag="r")
                            nc.tensor.matmul(
                                psr_t[:], oT[:, ssl], wo_sb[:, jn * SC:(jn + 1) * SC],
                                start=True, stop=True,
                            )
                            nc.any.tensor_copy(rt[:, jn * SC:(jn + 1) * SC], psr_t[:])
                        nc.sync.dma_start(out.ap()[ssl, :], rt[:])

    nc.compile()
    return nc


def _rope_tables():
    inv_freq = 1.0 / (THETA ** (np.arange(0, DK, 2, dtype=np.float64) / DK))  # [32]
    pos = np.arange(S, dtype=np.float64)
    freqs = pos[:, None] * inv_freq[None, :]      # [S, 32]
    cosT = np.cos(freqs).T.astype(np.float32)     # [32, S]
    sinT = np.sin(freqs).T.astype(np.float32)
    # t1 rows (per 64-block): [cos; cos];   t2s rows: [+sin; -sin]
    t1 = np.tile(np.concatenate([cosT, cosT], axis=0), (2, 1))      # [128, S]
    t2s_ = np.tile(np.concatenate([sinT, -sinT], axis=0), (2, 1))   # [128, S]
    return np.ascontiguousarray(t1), np.ascontiguousarray(t2s_)


def _host_prep(x, wq, wk, wv, wo):
    x2 = np.asarray(x, dtype=np.float32).reshape(S, D)
    xT = np.ascontiguousarray(x2.T)

    # even/odd de-interleave permutation within each head's 64 rows
    perm64 = np.concatenate([np.arange(0, DK, 2), np.arange(1, DK, 2)])
    perm128 = np.concatenate([perm64, perm64 + DK])

    t1, t2s_ = _rope_tables()
    trimask = np.triu(np.ones((P, P), dtype=np.float32))
    swp = np.zeros((P, P), dtype=np.float32)
    for b in range(2):
        for i in range(32):
            swp[b * 64 + i, b * 64 + 32 + i] = 1.0
            swp[b * 64 + 32 + i, b * 64 + i] = 1.0

    wq = np.asarray(wq, dtype=np.float32)
    wk = np.asarray(wk, dtype=np.float32)
    wv = np.asarray(wv, dtype=np.float32)
    wo = np.asarray(wo, dtype=np.float32)

    in_maps = []
    for c in range(NCORES):
        rows = slice(P * c, P * (c + 1))
        wq_c = wq[rows][perm128]
        wk_c = wk[rows][perm128]
        wv_c = wv[rows]
        in_maps.append({
            "xT": xT,
            "wqT": np.ascontiguousarray(wq_c.T),
            "wkT": np.ascontiguousarray(wk_c.T),
            "wvT": np.ascontiguousarray(wv_c.T),
            "woT": np.ascontiguousarray(wo[:, rows].T),
            "t1": t1,
            "t2s": t2s_,
            "trimask": trimask,
            "swapmat": swp,
        })
    return in_maps


def _install_ntff_hook():
    """Register the axon NTFF profiling hook (missing antenv.axon_hooks shim)."""
    import sys
    import types
    import importlib

    try:
        import antenv.axon_hooks  # noqa: F401
        return
    except ImportError:
        pass
    try:
        import antenv
        boot = importlib.import_module("trn_agent_boot.trn_boot")
        mod = types.ModuleType("antenv.axon_hooks")
        state = {"hook": None}
        mod.set_axon_ntff_profile_hook = lambda h: state.update(hook=h)
        mod.get_axon_ntff_profile_hook = lambda: state["hook"]
        sys.modules["antenv.axon_hooks"] = mod
        antenv.axon_hooks = mod
        hook = boot._ntff_profile_via_ctypes("/opt/axon/libaxon_pjrt.so")
        mod.set_axon_ntff_profile_hook(hook)
    except Exception as e:  # profiling is best-effort
        print(f"ntff hook install failed: {e}")


def kernel(x, wq, wk, wv, wo):
    global LAST_EXEC_NS, LAST_RESULTS
    from concourse import bass_utils

    trace_requested = bool(int(os.environ.get("TRN_TRACE", "0")))
    if trace_requested:
        _install_ntff_hook()
        # artifact upload needs remote storage; stub it out in this sandbox
        bass_utils.upload_artifacts = lambda tmpdir: "local://" + str(tmpdir)

    if "nc" not in _cache:
        _cache["nc"] = _build_bass()
    nc = _cache["nc"]

    in_maps = _host_prep(x, wq, wk, wv, wo)
    res = bass_utils.run_bass_kernel_spmd(
        nc, in_maps, core_ids=list(range(NCORES)), trace=trace_requested
    )
    LAST_EXEC_NS = res.exec_time_ns
    LAST_RESULTS = res
    acc = np.zeros((S, D), dtype=np.float32)
    for r in res.results:
        acc += np.asarray(r["out"], dtype=np.float32)
    return acc.reshape(1, S, D)


# revision 11
# speedup vs baseline: 1.7205x; 1.1783x over previous
"""Trainium2 Bass kernel for multi-head self-attention with RoPE (causal).

Problem shape (hardcoded): x [1, 4096, 1024], 16 heads, d_k=64, fp32.
Sharding: tensor-parallel over heads -- 2 heads per NeuronCore, 8 cores.
Each core computes Q/K/V projections for its 2 heads, RoPE, causal
attention, and a full [4096, 1024] partial of the output projection
(columns of wo matching its heads). Partials are summed on the host
(the all-reduce of row-parallel linear, done at unshard time).

Layout choices inside a core:
  - x is fed pre-transposed (xT [1024, 4096]) so all projection matmuls
    contract d on the partition axis without on-device transposes.
  - Q,K live as [128, 4096] = (2 heads x 64 dims) x seq, i.e. transposed.
    The even/odd RoPE interleave is pre-permuted into the wq/wk rows on
    the host so rotation pairs become contiguous 32-row blocks.
  - Scores are computed transposed, ST[k, q], so the post-exp P matrix
    feeds P@V directly as the moving operand with k on partitions.
  - V is computed in [seq, d] layout with an extra ones column per head:
    the P@V matmul then yields softmax numerators AND denominators.
  - All matmuls use float32r (1-pass fp22 multiply, fp32 accumulate).
"""

import os
import numpy as np

S = 4096
D = 1024
P = 128
DK = 64
SC = 512          # q-chunk width for attention
NQ = S // SC      # 8
NSUB = SC // P    # 4
NKC = S // P      # 32
PO = D // P       # 8 contraction chunks for projections
NCORES = 8
THETA = 10000.0

LAST_EXEC_NS = None
LAST_RESULTS = None

_cache = {}


def _build_bass():
    import concourse.bacc as bacc
    import concourse.tile as tile
    from concourse import mybir

    F32 = mybir.dt.float32
    F32R = mybir.dt.float32r
    BF16 = mybir.dt.bfloat16
    EXP = mybir.ActivationFunctionType.Exp
    LN = mybir.ActivationFunctionType.Ln
    MULT = mybir.AluOpType.mult
    ADD = mybir.AluOpType.add

    nc = bacc.Bacc("TRN2", target_bir_lowering=False, debug=False)

    xT = nc.dram_tensor("xT", [D, S], F32R, kind="ExternalInput")
    wqT = nc.dram_tensor("wqT", [D, P], F32R, kind="ExternalInput")
    wkT = nc.dram_tensor("wkT", [D, P], F32R, kind="ExternalInput")
    wvT = nc.dram_tensor("wvT", [D, P], F32R, kind="ExternalInput")
    woT = nc.dram_tensor("woT", [P, D], F32R, kind="ExternalInput")
    t1 = nc.dram_tensor("t1", [P, S], F32, kind="ExternalInput")
    t2s = nc.dram_tensor("t2s", [P, S], F32, kind="ExternalInput")
    trimask = nc.dram_tensor("trimask", [P, P], F32, kind="ExternalInput")
    swapmat = nc.dram_tensor("swapmat", [P, P], F32R, kind="ExternalInput")
    out = nc.dram_tensor("out", [S, D], F32, kind="ExternalOutput")

    xT_t = xT.ap().rearrange("(po pi) s -> pi po s", pi=P)
    wqT_t = wqT.ap().rearrange("(po pi) m -> pi po m", pi=P)
    wkT_t = wkT.ap().rearrange("(po pi) m -> pi po m", pi=P)
    wvT_t = wvT.ap().rearrange("(po pi) m -> pi po m", pi=P)

    with tile.TileContext(nc) as tc:
        with (
            tc.tile_pool(name="persist", bufs=1) as pp,
            tc.tile_pool(name="weights", bufs=1) as wp,
        ):
            QTr = pp.tile([P, S], F32R, tag="qtr")
            KTr0 = pp.tile([P, S], F32R, tag="ktr0")
            KTr1 = pp.tile([P, S], F32R, tag="ktr1")
            Vp = pp.tile([P, NKC, 130], BF16, tag="vp")
            oT = pp.tile([P, S], F32R, tag="ot")

            wq_sb = wp.tile([P, PO, P], F32R, tag="wq")
            wk_sb = wp.tile([P, PO, P], F32R, tag="wk")
            wv_sb = wp.tile([P, PO, P], F32R, tag="wv")
            wo_sb = wp.tile([P, D], F32R, tag="wo")
            tri_sb = wp.tile([P, P], F32, tag="tri")
            swap_sb = wp.tile([P, P], F32R, tag="swap")

            nc.sync.dma_start(wq_sb[:], wqT_t)
            nc.sync.dma_start(wk_sb[:], wkT_t)
            nc.sync.dma_start(wv_sb[:], wvT_t)
            nc.sync.dma_start(wo_sb[:], woT.ap())
            nc.sync.dma_start(tri_sb[:], trimask.ap())
            nc.sync.dma_start(swap_sb[:], swapmat.ap())

            # ones columns for the softmax-denominator trick
            # (memset can't write f32r -> memset an f32 tile, copy-convert)
            ones_sb = wp.tile([P, NKC], F32, tag="ones")
            nc.vector.memset(ones_sb[:], 1.0)
            nc.vector.memset(KTr0.bitcast(F32)[DK:P, :], 0.0)
            nc.vector.memset(KTr1.bitcast(F32)[0:DK, :], 0.0)
            nc.vector.tensor_copy(Vp[:, :, 64], ones_sb[:])
            nc.vector.tensor_copy(Vp[:, :, 129], ones_sb[:])

            # ---------------- Phase 1: projections + RoPE + V ----------------
            with (
                tc.tile_pool(name="ps1", bufs=1, space="PSUM") as ps1,
                tc.tile_pool(name="psv", bufs=2, space="PSUM") as psv_pool,
                tc.tile_pool(name="xchunk", bufs=2) as xpool,
                tc.tile_pool(name="tchunk", bufs=2) as tpool,
                tc.tile_pool(name="rope", bufs=2) as rpool,
            ):
                for j in range(NQ):
                    sl = slice(j * SC, (j + 1) * SC)
                    xt = xpool.tile([P, PO, SC], F32R, tag="xt")
                    nc.sync.dma_start(xt[:], xT_t[:, :, sl])
                    t1t = tpool.tile([P, SC], F32, tag="t1")
                    nc.sync.dma_start(t1t[:], t1.ap()[:, sl])
                    t2t = tpool.tile([P, SC], F32, tag="t2")
                    nc.sync.dma_start(t2t[:], t2s.ap()[:, sl])

                    for w_sb, dest, tagp in ((wq_sb, QTr, "q"), (wk_sb, None, "k")):
                        psq = ps1.tile([P, SC], F32, tag=tagp)
                        for po in range(PO):
                            nc.tensor.matmul(
                                psq[:], w_sb[:, po, :], xt[:, po, :],
                                start=(po == 0), stop=(po == PO - 1),
                            )
                        # RoPE: dest = t1*psq + swap(t2s*psq)
                        b = rpool.tile([P, SC], F32R, tag="b")
                        nc.vector.tensor_tensor(b[:], t2t[:], psq[:], MULT)
                        pssw = ps1.tile([P, SC], F32, tag="sw")
                        nc.tensor.matmul(pssw[:], swap_sb[:], b[:], start=True, stop=True)
                        a1 = rpool.tile([P, SC], F32, tag="a1")
                        nc.vector.tensor_tensor(a1[:], t1t[:], psq[:], MULT)
                        if dest is not None:
                            nc.vector.tensor_tensor(dest[:, sl], a1[:], pssw[:], ADD)
                        else:
                            nc.vector.tensor_tensor(
                                KTr0[0:DK, sl], a1[0:DK, :], pssw[0:DK, :], ADD)
                            nc.vector.tensor_tensor(
                                KTr1[DK:P, sl], a1[DK:P, :], pssw[DK:P, :], ADD)

                    for m in range(NSUB):
                        kc = j * NSUB + m
                        psv = psv_pool.tile([P, P], F32, tag="v")
                        for po in range(PO):
                            nc.tensor.matmul(
                                psv[:], xt[:, po, m * P:(m + 1) * P], wv_sb[:, po, :],
                                start=(po == 0), stop=(po == PO - 1),
                            )
                        nc.any.tensor_copy(Vp[:, kc, 0:64], psv[0:P, 0:64])
                        nc.any.tensor_copy(Vp[:, kc, 65:129], psv[0:P, 64:128])

            # ---------------- Phase 2+3: attention + projection ----------------
            with (
                tc.tile_pool(name="pst", bufs=3, space="PSUM") as pst,
                tc.tile_pool(name="pso", bufs=3, space="PSUM") as pso,
                tc.tile_pool(name="psr", bufs=2, space="PSUM") as psr,
                tc.tile_pool(name="ppool", bufs=3) as ppool,
                tc.tile_pool(name="small", bufs=2) as small,
                tc.tile_pool(name="rbp", bufs=2) as rbp,
                tc.tile_pool(name="rtp", bufs=2) as rtp,
            ):
                for jq in range(NQ):
                    qsl = slice(jq * SC, (jq + 1) * SC)
                    for h in range(2):
                        KTr_h = KTr0 if h == 0 else KTr1
                        pso_t = pso.tile([65, SC], F32, tag="o")
                        nkc = NSUB * (jq + 1)
                        for kc in range(nkc):
                            t = kc - NSUB * jq  # >= 0 -> diagonal straddle
                            col0 = t * P if t >= 0 else 0
                            ps_s = pst.tile([P, SC], F32, tag="st")
                            nc.tensor.matmul(
                                ps_s[:, col0:SC],
                                KTr_h[:, kc * P:(kc + 1) * P],
                                QTr[:, jq * SC + col0:(jq + 1) * SC],
                                start=True, stop=True,
                            )
                            pt = ppool.tile([P, SC], BF16, tag="p")
                            nc.scalar.activation(
                                pt[:, col0:SC], ps_s[:, col0:SC], EXP, scale=0.125
                            )
                            if t >= 0:
                                # diagonal block: zero keys above the diagonal
                                nc.vector.tensor_tensor(
                                    pt[:, col0:col0 + P], pt[:, col0:col0 + P],
                                    tri_sb[:], MULT,
                                )
                            nc.tensor.matmul(
                                pso_t[:, col0:SC],
                                Vp[:, kc, 65 * h:65 * h + 65],
                                pt[:, col0:SC],
                                start=(kc == 0), stop=(kc == nkc - 1),
                            )
                        # softmax denominators: fast DVE reciprocal, broadcast
                        # (custom DVE op needs an SBUF source -> ACT copy first)
                        dn_t = small.tile([1, SC], F32, tag="dn")
                        nc.scalar.activation(
                            dn_t[:], pso_t[64:65, :],
                            mybir.ActivationFunctionType.Copy,
                        )
                        rr_t = small.tile([1, SC], F32, tag="rr")
                        nc.vector.reciprocal_approx_fast(rr_t[:], dn_t[:])
                        rb_t = rbp.tile([DK, SC], F32, tag="rb")
                        nc.gpsimd.partition_broadcast(rb_t[:], rr_t[:])
                        nc.vector.tensor_tensor(
                            oT[DK * h:DK * (h + 1), qsl], pso_t[0:DK, :], rb_t[:], MULT
                        )
                    # output projection for the finished q-chunk
                    for m in range(NSUB):
                        sc = jq * NSUB + m
                        ssl = slice(sc * P, (sc + 1) * P)
                        rt = rtp.tile([P, D], F32, tag="rt")
                        for jn in range(2):
                            psr_t = psr.tile([P, SC], F32, tag="r")
                            nc.tensor.matmul(
                                psr_t[:], oT[:, ssl], wo_sb[:, jn * SC:(jn + 1) * SC],
                                start=True, stop=True,
                            )
                            nc.any.tensor_copy(rt[:, jn * SC:(jn + 1) * SC], psr_t[:])
                        nc.sync.dma_start(out.ap()[ssl, :], rt[:])

    nc.compile()
    return nc


def _rope_tables():
    inv_freq = 1.0 / (THETA ** (np.arange(0, DK, 2, dtype=np.float64) / DK))  # [32]
    pos = np.arange(S, dtype=np.float64)
    freqs = pos[:, None] * inv_freq[None, :]      # [S, 32]
    cosT = np.cos(freqs).T.astype(np.float32)     # [32, S]
    sinT = np.sin(freqs).T.astype(np.float32)
    # t1 rows (per 64-block): [cos; cos];   t2s rows: [+sin; -sin]
    t1 = np.tile(np.concatenate([cosT, cosT], axis=0), (2, 1))      # [128, S]
    t2s_ = np.tile(np.concatenate([sinT, -sinT], axis=0), (2, 1))   # [128, S]
    return np.ascontiguousarray(t1), np.ascontiguousarray(t2s_)


def _host_prep(x, wq, wk, wv, wo):
    x2 = np.asarray(x, dtype=np.float32).reshape(S, D)
    xT = np.ascontiguousarray(x2.T)

    # even/odd de-interleave permutation within each head's 64 rows
    perm64 = np.concatenate([np.arange(0, DK, 2), np.arange(1, DK, 2)])
    perm128 = np.concatenate([perm64, perm64 + DK])

    t1, t2s_ = _rope_tables()
    trimask = np.triu(np.ones((P, P), dtype=np.float32))
    swp = np.zeros((P, P), dtype=np.float32)
    for b in range(2):
        for i in range(32):
            swp[b * 64 + i, b * 64 + 32 + i] = 1.0
            swp[b * 64 + 32 + i, b * 64 + i] = 1.0

    wq = np.asarray(wq, dtype=np.float32)
    wk = np.asarray(wk, dtype=np.float32)
    wv = np.asarray(wv, dtype=np.float32)
    wo = np.asarray(wo, dtype=np.float32)

    in_maps = []
    for c in range(NCORES):
        rows = slice(P * c, P * (c + 1))
        wq_c = wq[rows][perm128]
        wk_c = wk[rows][perm128]
        wv_c = wv[rows]
        in_maps.append({
            "xT": xT,
            "wqT": np.ascontiguousarray(wq_c.T),
            "wkT": np.ascontiguousarray(wk_c.T),
            "wvT": np.ascontiguousarray(wv_c.T),
            "woT": np.ascontiguousarray(wo[:, rows].T),
            "t1": t1,
            "t2s": t2s_,
            "trimask": trimask,
            "swapmat": swp,
        })
    return in_maps


def _install_ntff_hook():
    """Register the axon NTFF profiling hook (missing antenv.axon_hooks shim)."""
    import sys
    import types
    import importlib

    try:
        import antenv.axon_hooks  # noqa: F401
        return
    except ImportError:
        pass
    try:
        import antenv
        boot = importlib.import_module("trn_agent_boot.trn_boot")
        mod = types.ModuleType("antenv.axon_hooks")
        state = {"hook": None}
        mod.set_axon_ntff_profile_hook = lambda h: state.update(hook=h)
        mod.get_axon_ntff_profile_hook = lambda: state["hook"]
        sys.modules["antenv.axon_hooks"] = mod
        antenv.axon_hooks = mod
        hook = boot._ntff_profile_via_ctypes("/opt/axon/libaxon_pjrt.so")
        mod.set_axon_ntff_profile_hook(hook)
    except Exception as e:  # profiling is best-effort
        print(f"ntff hook install failed: {e}")


def kernel(x, wq, wk, wv, wo):
    global LAST_EXEC_NS, LAST_RESULTS
    from concourse import bass_utils

    trace_requested = bool(int(os.environ.get("TRN_TRACE", "0")))
    if trace_requested:
        _install_ntff_hook()
        # artifact upload needs remote storage; stub it out in this sandbox
        bass_utils.upload_artifacts = lambda tmpdir: "local://" + str(tmpdir)

    if "nc" not in _cache:
        _cache["nc"] = _build_bass()
    nc = _cache["nc"]

    in_maps = _host_prep(x, wq, wk, wv, wo)
    res = bass_utils.run_bass_kernel_spmd(
        nc, in_maps, core_ids=list(range(NCORES)), trace=trace_requested
    )
    LAST_EXEC_NS = res.exec_time_ns
    LAST_RESULTS = res
    acc = np.zeros((S, D), dtype=np.float32)
    for r in res.results:
        acc += np.asarray(r["out"], dtype=np.float32)
    return acc.reshape(1, S, D)
